# revision 12
# baseline (speedup 1.0000x reference)
"""CRF (hidden2tag + Viterbi decode) Trainium2 kernel.

Device (8 NeuronCores, SPMD over the T axis): the memory-bound
emissions matmul  emis[t,k] = sum_h feats[t,h] * W[k,h]  over
feats [32768, 1024] f32 (128 MB streamed from HBM).

Host: bias add + the sequential Viterbi recurrence (T steps over a
K=5 state) in f32, bit-exact to the jax reference semantics.  The
recurrence is O(T*K^2) scalar work (~0.3 MFLOP) with a serial
dependence chain and f32 magnitudes ~4.6e4 whose argmax decisions sit
at the f32 quantization scale — any reassociated/parallel evaluation
flips near-tie backpointers, so it is run exactly sequentially.
"""

import numpy as np

T = 32768
H = 1024
K = 5
N_CORES = 8
T_CORE = T // N_CORES  # 4096
TILE_T = 512           # rows per device tile (2 MB per DMA)
N_TILES = T_CORE // TILE_T
START_IDX = 3
STOP_IDX = 4

_CACHE = {}


def _build_bass():
    import concourse.mybir as mybir
    from concourse import bacc
    from concourse.tile import TileContext
    from concourse.masks import make_identity

    f32 = mybir.dt.float32
    nc = bacc.Bacc("TRN2", target_bir_lowering=False)
    feats = nc.declare_dram_parameter("feats", [T_CORE, H], f32, isOutput=False)
    wt = nc.declare_dram_parameter("wt", [H, K], f32, isOutput=False)
    emis = nc.declare_dram_parameter("emis", [K, T_CORE], f32, isOutput=True)

    # t = n*TILE_T + p*4 + q : each partition reads 4 contiguous rows (16 KB)
    feats_r = feats.rearrange("(n p q) h -> n p (q h)", p=128, q=4)
    wt_r = wt.rearrange("(c p) k -> p c k", p=128)

    with TileContext(nc) as tc:
        with (
            tc.tile_pool(name="const", bufs=1) as cpool,
            tc.tile_pool(name="ft", bufs=3) as ftpool,
            tc.tile_pool(name="ftt", bufs=3) as fttpool,
            tc.tile_pool(name="eo", bufs=2) as eopool,
            tc.tile_pool(name="ptr", bufs=4, space="PSUM") as ptr,
            tc.tile_pool(name="pmm", bufs=2, space="PSUM") as pmm,
        ):
            ident = cpool.tile([128, 128], f32)
            make_identity(nc, ident)
            wt_sb = cpool.tile([128, 8, K], f32)
            nc.sync.dma_start(out=wt_sb, in_=wt_r)


            for n in range(N_TILES):
                ft = ftpool.tile([128, 4 * H], f32, tag="ft")
                nc.sync.dma_start(out=ft, in_=feats_r[n])
                eps = pmm.tile([K, TILE_T], f32, tag="eps")
                for hc in range(H // 128):
                    ftt = fttpool.tile([128, TILE_T], f32, tag="ftt")
                    use_act = (n * (H // 128) + hc) % 2 == 0
                    for q in range(4):
                        ps = ptr.tile([128, 128], f32, tag="ps")
                        nc.tensor.transpose(
                            ps, ft[:, q * H + hc * 128 : q * H + (hc + 1) * 128], ident
                        )
                        if use_act:
                            nc.scalar.copy(ftt[:, q * 128 : (q + 1) * 128], ps)
                        else:
                            nc.vector.tensor_copy(ftt[:, q * 128 : (q + 1) * 128], ps)
                    nc.tensor.matmul(
                        eps,
                        lhsT=wt_sb[:, hc, :],
                        rhs=ftt,
                        start=(hc == 0),
                        stop=(hc == H // 128 - 1),
                    )
                eo = eopool.tile([K, TILE_T], f32, tag="eo")
                nc.scalar.copy(eo, eps)
                nc.sync.dma_start(out=emis[:, n * TILE_T : (n + 1) * TILE_T], in_=eo)
    nc.compile()
    return nc


def _run_device(feats, W, trace=False):
    from concourse.bass_utils import run_bass_kernel_spmd

    if "nc" not in _CACHE:
        _CACHE["nc"] = _build_bass()
    nc = _CACHE["nc"]

    wt = np.ascontiguousarray(np.asarray(W, np.float32).T)  # [H, K]
    f = np.asarray(feats, np.float32).reshape(T, H)
    in_maps = [
        {"feats": np.ascontiguousarray(f[c * T_CORE : (c + 1) * T_CORE]), "wt": wt}
        for c in range(N_CORES)
    ]
    res = run_bass_kernel_spmd(nc, in_maps, list(range(N_CORES)), trace=trace)
    emis = np.empty((T, K), np.float32)
    for c in range(N_CORES):
        e = res.results[c]["emis"]  # [K, T_CORE], cols (n, q, p); t = n*512+p*4+q
        emis[c * T_CORE : (c + 1) * T_CORE] = (
            e.reshape(K, N_TILES, 4, 128).transpose(1, 3, 2, 0).reshape(T_CORE, K)
        )
    return emis, res


def _viterbi_host(emissions, transitions):
    """Bit-exact f32 emulation of the reference lax.scan Viterbi."""
    trans = np.asarray(transitions, np.float32)
    v = np.full(K, np.float32(-10000.0), np.float32)
    v[START_IDX] = np.float32(0.0)
    bptrs = np.empty((T, K), np.int32)
    for t in range(T):
        ntv = v[None, :] + trans        # [next, prev]
        bptrs[t] = ntv.argmax(1)
        v = ntv.max(1) + emissions[t]
    terminal = v + trans[STOP_IDX]
    best = int(terminal.argmax())
    score = terminal[best]
    path = np.empty(T, np.int32)
    tag = best
    for t in range(T - 1, -1, -1):
        path[t] = tag
        tag = bptrs[t, tag]
    return np.float32(score), path


def kernel(feats, W, b, transitions):
    emis_dev, _ = _run_device(feats, W)
    emissions = emis_dev + np.asarray(b, np.float32)[None, :]
    score, path = _viterbi_host(emissions, transitions)
    return score, path


# revision 21
# speedup vs baseline: 1.7019x; 1.7019x over previous
"""CRF (hidden2tag + Viterbi decode) Trainium2 kernel.

Device (8 NeuronCores, SPMD over the T axis): the memory-bound
emissions matmul  emis[t,k] = sum_h feats[t,h] * W[k,h]  over
feats [32768, 1024] f32 (128 MB streamed from HBM).  Each core gets
its T-shard pre-transposed on the host to [H, T_CORE] so the PE
contracts over h directly from DMA-friendly contiguous tiles — no
on-chip transposes.

Host: bias add + the sequential Viterbi recurrence (T steps over a
K=5 state) in f32, bit-exact to the jax reference semantics.  The
recurrence is O(T*K^2) scalar work (~0.3 MFLOP) with a serial
dependence chain and f32 magnitudes ~4.6e4 whose argmax decisions sit
at the f32 quantization scale — any reassociated/parallel evaluation
(and any reduced-precision matmul: float32r flips 2 path elements)
breaks bit-exactness, so the scan runs sequentially and the matmul
stays full fp32.
"""

import numpy as np

T = 32768
H = 1024
K = 5
N_CORES = 8
T_CORE = T // N_CORES  # 4096
TILE_T = 512           # columns per PSUM accumulator bank
N_TILES = T_CORE // TILE_T
START_IDX = 3
STOP_IDX = 4

_CACHE = {}


def _build_bass():
    import concourse.mybir as mybir
    from concourse import bacc
    from concourse.tile import TileContext

    f32 = mybir.dt.float32
    nc = bacc.Bacc("TRN2", target_bir_lowering=False)
    featsT = nc.declare_dram_parameter("featsT", [H, T_CORE], f32, isOutput=False)
    wt = nc.declare_dram_parameter("wt", [H, K], f32, isOutput=False)
    emis = nc.declare_dram_parameter("emis", [K, T_CORE], f32, isOutput=True)

    wt_r = wt.rearrange("(c p) k -> p c k", p=128)
    n_hc = H // 128

    with TileContext(nc) as tc:
        with (
            tc.tile_pool(name="const", bufs=1) as cpool,
            tc.tile_pool(name="ftb", bufs=3) as ftpool,
            tc.tile_pool(name="eo", bufs=2) as eopool,
            tc.tile_pool(name="pmm", bufs=1, space="PSUM") as pmm,
        ):
            wt_sb = cpool.tile([128, n_hc, K], f32)
            nc.sync.dma_start(out=wt_sb, in_=wt_r)

            eps = []
            for j in range(N_TILES):
                eps_j = pmm.tile([K, TILE_T], f32, tag=f"eps{j}", name=f"eps{j}")
                eps.append(eps_j)
            for hc in range(n_hc):
                ftb = ftpool.tile([128, T_CORE], f32, tag="ftb")
                nc.sync.dma_start(out=ftb, in_=featsT[hc * 128 : (hc + 1) * 128, :])
                for j in range(N_TILES):
                    nc.tensor.matmul(
                        eps[j],
                        lhsT=wt_sb[:, hc, :],
                        rhs=ftb[:, j * TILE_T : (j + 1) * TILE_T],
                        start=(hc == 0),
                        stop=(hc == n_hc - 1),
                    )
            for j in range(N_TILES):
                eo = eopool.tile([K, TILE_T], f32, tag="eo")
                nc.scalar.copy(eo, eps[j])
                nc.sync.dma_start(out=emis[:, j * TILE_T : (j + 1) * TILE_T], in_=eo)
    nc.compile()
    return nc


def _run_device(feats, W, trace=False):
    from concourse.bass_utils import run_bass_kernel_spmd

    if "nc" not in _CACHE:
        _CACHE["nc"] = _build_bass()
    nc = _CACHE["nc"]

    wt = np.ascontiguousarray(np.asarray(W, np.float32).T)  # [H, K]
    f = np.asarray(feats, np.float32).reshape(T, H)
    in_maps = [
        {
            "featsT": np.ascontiguousarray(f[c * T_CORE : (c + 1) * T_CORE].T),
            "wt": wt,
        }
        for c in range(N_CORES)
    ]
    res = run_bass_kernel_spmd(nc, in_maps, list(range(N_CORES)), trace=trace)
    emis = np.empty((T, K), np.float32)
    for c in range(N_CORES):
        emis[c * T_CORE : (c + 1) * T_CORE] = res.results[c]["emis"].T
    return emis, res


def _viterbi_host(emissions, transitions):
    """Bit-exact f32 emulation of the reference lax.scan Viterbi."""
    trans = np.asarray(transitions, np.float32)
    v = np.full(K, np.float32(-10000.0), np.float32)
    v[START_IDX] = np.float32(0.0)
    bptrs = np.empty((T, K), np.int32)
    for t in range(T):
        ntv = v[None, :] + trans        # [next, prev]
        bptrs[t] = ntv.argmax(1)
        v = ntv.max(1) + emissions[t]
    terminal = v + trans[STOP_IDX]
    best = int(terminal.argmax())
    score = terminal[best]
    path = np.empty(T, np.int32)
    tag = best
    for t in range(T - 1, -1, -1):
        path[t] = tag
        tag = bptrs[t, tag]
    return np.float32(score), path


def kernel(feats, W, b, transitions):
    emis_dev, _ = _run_device(feats, W)
    emissions = emis_dev + np.asarray(b, np.float32)[None, :]
    score, path = _viterbi_host(emissions, transitions)
    return score, path
